# revision 11
# baseline (speedup 1.0000x reference)
"""Sigmoid-attention (DiffAttention) kernel for 8 Trainium2 NeuronCores.

Problem:  N=L=4096, H=8 heads, M=D=64.
    scores[n,l,h] = sigmoid(q[n,h,:] . k[l,h,:])
    out[n,h,:]    = (scores @ v) / sum_l(scores)        (per head)

Sharding: one head per core (8 heads == 8 cores).

Math: with u = tanh(x/2) = 2*sigmoid(x) - 1,
    sum_l sigmoid(x_l) v_l = 0.5*(sum_l v_l + sum_l u_l v_l)
    sum_l sigmoid(x_l)     = 0.5*(L + sum_l u_l)
so out = (corr + U@V) / (L + sum U), where corr = [L, sum_l v_l] is a
per-head constant, injected into the PSUM accumulator by two rank-1
PE matmuls (corr_hi/corr_lo fp16 split x a ones row) at chunk start.
The per-element nonlinearity is then u = tanh(x/2), an ODD function,
split across two engines (the tanh pass is the kernel bottleneck —
131072 PSUM columns at 1 col/lane-cycle on either engine):
  - ACT (Scalar) engine: native Tanh activation (scale=0.5), ~2/3
  - DVE (Vector) engine: two custom 8-stage DVE ops evaluating a
    clamped odd-polynomial composition p2(p1(clamp(x))) ~ tanh(x/2)
    (max err ~1e-3 incl the fp16 handoff), ~1/3

PE geometry (measured): matmul streaming is SBUF-line limited: a
128-row fp16 column costs one line (~0.76 ns); a 64-row column costs
half, so TWO 64-contraction streams on opposite row-group halves
sustain ~2x. Hence both matmuls use 64-contraction dual streams
(even l_tiles / Q-copy on partitions 0-63, odd on 64-127):
    mm1: S^T[l,n] = matmul(lhsT=K^T[:,l_tile], rhs=Q^T[:,n_chunk])
    u:   aT = tanh(S^T/2)  (ACT or DVE per a static pattern)
    mm2: acc_a[65,n] += (v_tile rows 0:64)^T @ aT[0:64, :]     (T0)
         acc_b[65,n] += (v_tile rows 64:128)^T @ aT[64:128, :] (T8)
         h-split into 512-col halves so consecutive matmuls on the
         same accumulator touch disjoint PSUM addresses (same-address
         accumulation back-to-back serializes on instruction duration).
    out: s = acc_a + acc_b (DVE, PSUM->SBUF; GpSimd cannot read PSUM);
         out = s[1:65] * 1/s[0] (DVE reciprocal + GpSimd broadcast/
         mul). The ones column is FIRST in the packed V so the
         normalizer lands in partition 0 (custom-DVE and
         partition_broadcast sources must start at partition 0).

PSUM (8 banks x 2KB): 2 x sT (2 banks each) + acc_a + acc_b (2 each).
"""

from contextlib import ExitStack

import numpy as np

import concourse.bass as bass
import concourse.mybir as mybir
import concourse.tile as tile
from concourse import bacc
from concourse import dve_ops
from concourse.bass import ts
from concourse.bass_utils import run_bass_kernel_spmd
from concourse.dve_spec import (
    Spec,
    Src0,
    C0,
    C1,
    C2,
    C3,
    Zero,
    lower,
    maxx,
    minn,
    sq,
    _has_src1,
    _spill_c3_to_src1,
)
from concourse.dve_uop import DveOpSpec

N, L, H, M, D = 4096, 4096, 8, 64, 64
NCORES = 8
NCHUNK = 1024  # n columns per PSUM chunk
NCHUNKS = N // NCHUNK
LTILES = L // 128
VW = D + 1  # ones column + V columns
SKEW = 6  # mm2 trails mm1 by SKEW l_tiles so PE never waits on tanh
CDT = mybir.dt.float16  # PE input dtype
FP32 = mybir.dt.float32
TANH = mybir.ActivationFunctionType.Tanh
IDENT = mybir.ActivationFunctionType.Identity

# tanh(x/2) ~ p2(p1(clamp(x, +-B))); p1 odd deg-5, p2 odd deg-7.
# Minimax fit on [0, B] with the fp16 handoff in the loop; max err ~1e-3.
B_CLAMP = 8.0
A1, B1, C1_ = 5.80866609e-02, -8.99562531e-04, 6.82759119e-06
A2, B2, C2_, D2 = 8.56865936e00, -1.58991416e02, 2.36996893e03, -1.65801292e04

# l_tiles handled by the DVE path per chunk (10 of 32; ACT does the
# rest plus the epilogue acc_a copy, DVE adds acc_b + reciprocal).
_DVE_LT = frozenset((2, 5, 8, 11, 14, 18, 21, 24, 27, 30))


def _dve_tile(ci: int, lt: int) -> bool:
    return lt in _DVE_LT


_CACHE: dict = {}


def _register_dve_ops():
    """Register the two tanh-composition custom DVE ops at runtime.

    Rows 1..16 are taken by the production OPS; we take the next free
    rows and compute the uops sha the same way DveOp.compile does."""
    if "ops" in _CACHE:
        return _CACHE["ops"]

    def ref_p1(in0, in1, s0, s1, imm2):
        xc = np.clip(in0.astype(np.float64), s0, -s0)
        u = xc * xc
        return (((in1 * u + imm2) * u + s1) * xc).astype(np.float32)

    def ref_p2(in0, in1, s0, s1, imm2):
        y = in0.astype(np.float64)
        v = y * y
        return ((((in1 * v + imm2) * v + s1) * v + s0) * y).astype(np.float32)

    # p1: xc = clamp(Src0, C0, -C0); out = ((C3*xc^2 + C2)*xc^2 + C1)*xc
    xc = minn(maxx(Src0, C0), Zero - C0)
    u = sq(xc)
    body1 = _spill_c3_to_src1(((C3 * u + C2) * u + C1) * xc)
    # p2: v = Src0^2; out = (((C3*v + C2)*v + C1)*v + C0)*Src0
    v = sq(Src0)
    body2 = _spill_c3_to_src1((((C3 * v + C2) * v + C1) * v + C0) * Src0)

    made = []
    for name, body, ref in (
        ("TANH_HALF_P1_ANT", body1, ref_p1),
        ("TANH_HALF_P2_ANT", body2, ref_p2),
    ):
        if name not in dve_ops._SUB_OPCODE_FOR_NAME:
            row = dve_ops._CUSTOM_DVE_ROW_BASE + len(dve_ops.OPS)
            assert row < 0x20
            dve_ops._SUB_OPCODE_FOR_NAME[name] = row
            spec = Spec(body=body, reference=ref)
            sha = {
                ver: DveOpSpec(
                    name=name,
                    opcode=row,
                    uops=lower(spec, ver=ver),
                    rd1_en=_has_src1(spec),
                ).sha(ver)
                for ver in ("v3", "v4")
            }
            op = dve_ops.DveOp(name, spec, subdim=False, uops_sha=sha)
            dve_ops.OPS.append(op)
            dve_ops.CUSTOM_DVE_SPECS[name] = op.spec
        made.append(next(o for o in dve_ops.OPS if o.name == name))
    _CACHE["ops"] = tuple(made)
    return _CACHE["ops"]


def build_nc():
    op_p1, op_p2 = _register_dve_ops()
    nc = bacc.Bacc("TRN2", target_bir_lowering=False, debug=False)

    # q1/k1 hold Q^T/K^T once [64, N]; each is DMA'd to both SBUF
    # partition halves (the 64-contraction dual-stream needs 2 copies).
    q1_d = nc.dram_tensor("q1", [64, N], CDT, kind="ExternalInput").ap()
    k1_d = nc.dram_tensor("k1", [64, L], CDT, kind="ExternalInput").ap()
    v1_d = nc.dram_tensor("v1", [128, LTILES * VW], CDT, kind="ExternalInput").ap()
    # crow: [corr_hi(65) | corr_lo(65) | ones(NCHUNK)] on one partition
    crow_d = nc.dram_tensor("crow", [1, 2 * VW + NCHUNK], CDT, kind="ExternalInput").ap()
    # cst col0: C1_ ; col1: D2 (per-partition scalars for the DVE ops)
    cst_d = nc.dram_tensor("cst", [128, 2], FP32, kind="ExternalInput").ap()
    out_d = nc.dram_tensor("out", [D, N], FP32, kind="ExternalOutput").ap()

    with ExitStack() as ctx:
        tc = ctx.enter_context(tile.TileContext(nc))
        const = ctx.enter_context(tc.tile_pool(name="const", bufs=1))
        apool = ctx.enter_context(tc.tile_pool(name="apool", bufs=SKEW + 2))
        ypool = ctx.enter_context(tc.tile_pool(name="ypool", bufs=2))
        io = ctx.enter_context(tc.tile_pool(name="io", bufs=2))
        psA = ctx.enter_context(tc.tile_pool(name="psA", bufs=2, space="PSUM"))
        psAcc = ctx.enter_context(tc.tile_pool(name="psAcc", bufs=1, space="PSUM"))

        q2_s = const.tile([128, N], CDT)
        k2_s = const.tile([128, L], CDT)
        v1_s = const.tile([128, LTILES * VW], CDT)
        crow_s = const.tile([1, 2 * VW + NCHUNK], CDT)
        cst_s = const.tile([128, 2], FP32)
        nc.sync.dma_start(out=cst_s, in_=cst_d)
        nc.sync.dma_start(out=crow_s, in_=crow_d)
        # Split input loads so the first l_tiles / n-chunks unblock early.
        for ci in range(NCHUNKS):
            cs = ci * NCHUNK
            sl = slice(cs, cs + NCHUNK)
            nc.sync.dma_start(out=k2_s[0:64, sl], in_=k1_d[:, sl])
            nc.sync.dma_start(out=k2_s[64:128, sl], in_=k1_d[:, sl])
            nc.sync.dma_start(
                out=v1_s[:, ci * 8 * VW : (ci + 1) * 8 * VW],
                in_=v1_d[:, ci * 8 * VW : (ci + 1) * 8 * VW],
            )
            nc.sync.dma_start(out=q2_s[0:64, sl], in_=q1_d[:, sl])
            nc.sync.dma_start(out=q2_s[64:128, sl], in_=q1_d[:, sl])

        c1_ap = cst_s[:, 0:1]
        d2_ap = cst_s[:, 1:2]
        ch_ap = crow_s[0:1, 0:VW]
        cl_ap = crow_s[0:1, VW : 2 * VW]
        ones_ap = crow_s[0:1, 2 * VW : 2 * VW + NCHUNK]

        def mm1pair(ci, lt, sT_e, sT_o):
            # even l_tile on partitions 0-63, odd on 64-127; interleave
            # so the two row-group streams run concurrently on the PE.
            cs = ci * NCHUNK
            ke = k2_s[0:64, ts(lt, 128)]
            ko = k2_s[64:128, ts(lt + 1, 128)]
            for h in range(NCHUNK // 512):
                qsl = slice(cs + h * 512, cs + (h + 1) * 512)
                nc.tensor.matmul(
                    sT_e[:, ts(h, 512)], ke, q2_s[0:64, qsl], start=True, stop=True
                )
                nc.tensor.matmul(
                    sT_o[:, ts(h, 512)], ko, q2_s[64:128, qsl], start=True, stop=True
                )

        def tanh_u(ci, lt, sT):
            """aT = tanh(sT/2), on ACT or DVE per the static pattern."""
            aT = apool.tile([128, NCHUNK], CDT, tag="aT", name="aT")
            if _dve_tile(ci, lt):
                y1 = ypool.tile([128, NCHUNK], CDT, tag="y1", name="y1")
                nc.vector._custom_dve(
                    op_p1, out=y1, in0=sT, in1=c1_ap, s0=-B_CLAMP, s1=A1, imm2=B1
                )
                nc.vector._custom_dve(
                    op_p2, out=aT, in0=y1, in1=d2_ap, s0=A2, s1=B2, imm2=C2_
                )
            else:
                nc.scalar.activation(aT, sT, TANH, scale=0.5)
            return aT

        def mm2(lt, aT, acc_a, acc_b):
            # 64-contraction dual streams; interleave a/b and h-split so
            # same-accumulator matmuls touch disjoint columns.
            va = v1_s[0:64, lt * VW : (lt + 1) * VW]
            vb = v1_s[64:128, lt * VW : (lt + 1) * VW]
            last = lt == LTILES - 1
            for h in range(NCHUNK // 512):
                hs = ts(h, 512)
                nc.tensor.matmul(acc_a[:, hs], va, aT[0:64, hs], start=False, stop=last)
                nc.tensor.matmul(
                    acc_b[:, hs], vb, aT[64:128, hs], start=lt == 0, stop=last
                )

        for ci in range(NCHUNKS):
            cs = ci * NCHUNK
            acc_a = psAcc.tile([VW, NCHUNK], FP32, tag="acc_a")
            acc_b = psAcc.tile([VW, NCHUNK], FP32, tag="acc_b")
            # Seed acc_a with the corr constant: rank-1 matmuls (fp16
            # hi/lo split of [L, sum_l v_l]) against a ones row, in
            # 512-col pieces (single PSUM bank per matmul).
            for h in range(NCHUNK // 512):
                hs = ts(h, 512)
                ones_h = crow_s[0:1, 2 * VW + h * 512 : 2 * VW + (h + 1) * 512]
                nc.tensor.matmul(acc_a[:, hs], ch_ap, ones_h, start=True, stop=False)
                nc.tensor.matmul(acc_a[:, hs], cl_ap, ones_h, start=False, stop=False)
            aTs = [None] * LTILES

            def mm1sig(ci, lt):
                sT_e = psA.tile([128, NCHUNK], FP32, tag="sT", name="sT")
                sT_o = psA.tile([128, NCHUNK], FP32, tag="sT", name="sT")
                mm1pair(ci, lt, sT_e, sT_o)
                aTs[lt] = tanh_u(ci, lt, sT_e)
                aTs[lt + 1] = tanh_u(ci, lt + 1, sT_o)

            for lt in range(0, SKEW, 2):
                mm1sig(ci, lt)
            for lt in range(SKEW, LTILES, 2):
                # mm2 first: its tanh is SKEW iterations old, so PE never
                # stalls here; any wait lands on mm1 (sT slot).
                mm2(lt - SKEW, aTs[lt - SKEW], acc_a, acc_b)
                mm2(lt - SKEW + 1, aTs[lt - SKEW + 1], acc_a, acc_b)
                aTs[lt - SKEW] = aTs[lt - SKEW + 1] = None
                mm1sig(ci, lt)
            for lt in range(LTILES - SKEW, LTILES):
                mm2(lt, aTs[lt], acc_a, acc_b)
                aTs[lt] = None

            # Epilogue: s = acc_a + acc_b (corr already inside acc_a);
            # out = s[1:65] / s[0]. An engine instruction may read only
            # ONE operand from PSUM, so ACT Identity-copies acc_a to
            # SBUF and DVE adds acc_b.
            sa = io.tile([VW, NCHUNK], FP32, tag="sa")
            nc.scalar.activation(sa, acc_a, IDENT)
            s = io.tile([VW, NCHUNK], FP32, tag="s")
            nc.vector.tensor_add(s, sa, acc_b)
            rec = io.tile([1, NCHUNK], FP32, tag="rec")
            nc.vector.reciprocal_approx_fast(out=rec, in_=s[0:1, :])
            bc = io.tile([VW, NCHUNK], FP32, tag="bc")
            nc.gpsimd.partition_broadcast(bc, rec, channels=VW)
            o = io.tile([VW, NCHUNK], FP32, tag="o")
            nc.gpsimd.tensor_mul(o, s, bc)
            nc.sync.dma_start(out=out_d[:, cs : cs + NCHUNK], in_=o[1:VW, :])

    nc.compile()
    return nc


def get_nc():
    if "nc" not in _CACHE:
        _CACHE["nc"] = build_nc()
    return _CACHE["nc"]


def make_in_maps(queries, keys, values):
    np_cdt = mybir.dt.np(CDT)
    in_maps = []
    for h in range(NCORES):
        qT = np.ascontiguousarray(queries[:, h, :].T.astype(np_cdt))
        kT = np.ascontiguousarray(keys[:, h, :].T.astype(np_cdt))
        v1 = np.empty((L, VW), np_cdt)
        v1[:, 0] = 1.0  # ones column FIRST: normalizer in acc partition 0
        v1[:, 1:] = values[:, h, :]
        v1p = np.ascontiguousarray(
            v1.reshape(LTILES, 128, VW).transpose(1, 0, 2).reshape(128, LTILES * VW)
        )
        corr = np.zeros(VW, np.float64)
        corr[0] = float(L)
        corr[1:] = values[:, h, :].astype(np.float64).sum(axis=0)
        ch = corr.astype(np_cdt)
        cl = (corr - ch.astype(np.float64)).astype(np_cdt)
        crow = np.zeros((1, 2 * VW + NCHUNK), np_cdt)
        crow[0, :VW] = ch
        crow[0, VW : 2 * VW] = cl
        crow[0, 2 * VW :] = 1.0
        cst = np.zeros((128, 2), np.float32)
        cst[:, 0] = C1_
        cst[:, 1] = D2
        in_maps.append({"q1": qT, "k1": kT, "v1": v1p, "crow": crow, "cst": cst})
    return in_maps


def run(queries, keys, values, trace=False):
    """Returns (out [N,H,D] fp32, BassKernelResults)."""
    nc = get_nc()
    in_maps = make_in_maps(queries, keys, values)
    res = run_bass_kernel_spmd(nc, in_maps, core_ids=list(range(NCORES)), trace=trace)
    out = np.empty((N, H, D), np.float32)
    for h in range(NCORES):
        out[:, h, :] = res.results[h]["out"].T
    return out, res


def kernel(queries, keys, values):
    out, _ = run(np.asarray(queries), np.asarray(keys), np.asarray(values))
    return out
